# revision 11
# baseline (speedup 1.0000x reference)
"""Trainium2 Bass kernel for cross-attention (single query per position, m=16 context).

Reference computation (per batch b, position n):
  q = x @ W_q                      [n, 512] -> heads h=8, d=64
  k,v = y @ W_kv                   [n, m, 512] each
  dots[h,m] = (q_h . k_mh) / 8
  attn = softmax_m(dots)
  out = (sum_m attn * v) @ W_out + b_out

Sharding: data-parallel over batch (8 batches -> 8 NeuronCores), weights replicated.
"""

import numpy as np
from contextlib import ExitStack

import concourse.bass as bass
import concourse.bacc as bacc
import concourse.mybir as mybir
import concourse.tile as tile
from concourse.bass_utils import run_bass_kernel_spmd
from concourse.masks import make_identity

B, N, M, DIM = 8, 2048, 16, 256
HEADS, DHEAD, INNER = 8, 64, 512
SCALE = DHEAD**-0.5
NCORES = 8
T = 128          # positions per tile
NT = N // T      # 16 tiles per core

F32 = mybir.dt.float32
CD = mybir.dt.float16  # compute dtype: fp16 keeps 10 mantissa bits and 2x DVE modes


def _build_nc():
    nc = bacc.Bacc("TRN2", target_bir_lowering=False, debug=False, num_devices=NCORES)
    x = nc.dram_tensor("x", [N, DIM], F32, kind="ExternalInput").ap()
    y = nc.dram_tensor("y", [N * M, DIM], F32, kind="ExternalInput").ap()
    wq = nc.dram_tensor("wq", [DIM, INNER], F32, kind="ExternalInput").ap()
    wkv = nc.dram_tensor("wkv", [DIM, 2 * INNER], F32, kind="ExternalInput").ap()
    wout = nc.dram_tensor("wout", [INNER, DIM], F32, kind="ExternalInput").ap()
    bout = nc.dram_tensor("bout", [1, DIM], F32, kind="ExternalInput").ap()
    out = nc.dram_tensor("out", [N, DIM], F32, kind="ExternalOutput").ap()

    with tile.TileContext(nc) as tc:
        with ExitStack() as ctx:
            _body(ctx, tc, out, x, y, wq, wkv, wout, bout)
    nc.compile()
    return nc


def _body(ctx, tc, out, x, y, wq, wkv, wout, bout):
    nc = tc.nc
    consts = ctx.enter_context(tc.tile_pool(name="consts", bufs=1))
    stage = ctx.enter_context(tc.tile_pool(name="stage", bufs=2))
    ypool = ctx.enter_context(tc.tile_pool(name="ypool", bufs=2))
    ytp = ctx.enter_context(tc.tile_pool(name="ytp", bufs=1))
    work = ctx.enter_context(tc.tile_pool(name="work", bufs=2))
    scratch = ctx.enter_context(tc.tile_pool(name="scratch", bufs=1))
    tp_psum = ctx.enter_context(tc.tile_pool(name="tp_psum", bufs=2, space="PSUM"))
    kv_psum = ctx.enter_context(tc.tile_pool(name="kv_psum", bufs=2, space="PSUM"))
    q_psum = ctx.enter_context(tc.tile_pool(name="q_psum", bufs=1, space="PSUM"))
    o_psum = ctx.enter_context(tc.tile_pool(name="o_psum", bufs=1, space="PSUM"))

    ident = consts.tile([128, 128], F32, tag="ident")
    make_identity(nc, ident[:])
    ident_cd = consts.tile([128, 128], CD, tag="ident_cd")
    nc.any.tensor_copy(ident_cd[:], ident[:])

    # --- weights: [c, cols] with contraction chunked to 128 partitions ---
    def load_w(ap, n_chunks, cols, name):
        st = scratch.tile([128, n_chunks, cols], F32, tag="wstage")
        nc.sync.dma_start(st[:], ap.rearrange("(a p) i -> p a i", p=128))
        cd = consts.tile([128, n_chunks, cols], CD, tag=f"{name}_cd")
        nc.any.tensor_copy(cd[:], st[:])
        return cd

    wq_sb = load_w(wq, 2, INNER, "wq")
    wkv_sb = load_w(wkv, 2, 2 * INNER, "wkv")
    wout_sb = load_w(wout, 4, DIM, "wout")

    # bias: added to the out-proj psum via ones[1,128].T @ bout[1,256]
    bout_f = consts.tile([1, DIM], F32, tag="bout_f")
    nc.sync.dma_start(bout_f[:], bout)
    ones_sb = consts.tile([1, 128], CD, tag="ones")
    nc.any.memset(ones_sb[:], 1.0)
    bout_cd = consts.tile([1, DIM], CD, tag="bout_cd")
    nc.any.tensor_copy(bout_cd[:], bout_f[:])

    x_t = x.rearrange("(t p) c -> t p c", p=T)
    y_t = y.rearrange("(t p m) c -> t p m c", p=T, m=M)
    out_t = out.rearrange("(t p) c -> t p c", p=T)

    for t in range(NT):
        # ---- x tile -> xT chunks [c, pos] ----
        x_sb = stage.tile([T, DIM], F32, tag="x")
        nc.sync.dma_start(x_sb[:], x_t[t])
        xT_ps = tp_psum.tile([128, 2, 128], F32, tag="tp")
        for ci in range(2):
            nc.tensor.transpose(xT_ps[:, ci], x_sb[:, bass.ts(ci, 128)], ident[:])
        xT = work.tile([128, 2, 128], CD, tag="xT")
        nc.any.tensor_copy(xT[:], xT_ps[:])

        # ---- q projection ----
        q_ps = q_psum.tile([T, INNER], F32, tag="q")
        for ci in range(2):
            nc.tensor.matmul(q_ps[:], xT[:, ci], wq_sb[:, ci],
                             start=(ci == 0), stop=(ci == 1))
        q_cd = work.tile([T, INNER], CD, tag="q_cd")
        nc.any.tensor_copy(q_cd[:], q_ps[:])

        # ---- y tile -> yT blocks, 4 transposes per psum tile, wide evac ----
        y_sb = ypool.tile([T, M, DIM], F32, tag="y")
        nc.sync.dma_start(y_sb[:], y_t[t])
        yT = ytp.tile([128, M, 2, 128], CD, tag="yT")
        y_flat = y_sb[:].rearrange("p m c -> p (m c)")
        for j in range(8):  # j covers (m, ci) pairs 4j..4j+3
            ps = tp_psum.tile([128, 4, 128], F32, tag="tp")
            for u in range(4):
                blk = 4 * j + u
                nc.tensor.transpose(ps[:, u], y_flat[:, bass.ts(blk, 128)], ident[:])
            nc.any.tensor_copy(
                yT[:].rearrange("p m c2 f -> p (m c2) f")[:, 4 * j:4 * j + 4], ps[:])

        # ---- kv projection per m into one [128,1024] psum; single evac ----
        kv_sb = work.tile([T, M, 2 * INNER], CD, tag="kv")
        for m in range(M):
            kv_ps = kv_psum.tile([T, 2 * INNER], F32, tag="kv")
            for ci in range(2):
                nc.tensor.matmul(kv_ps[:, 0:INNER], yT[:, m, ci],
                                 wkv_sb[:, ci, 0:INNER],
                                 start=(ci == 0), stop=(ci == 1))
            for ci in range(2):
                nc.tensor.matmul(kv_ps[:, INNER:2 * INNER], yT[:, m, ci],
                                 wkv_sb[:, ci, INNER:2 * INNER],
                                 start=(ci == 0), stop=(ci == 1))
            nc.any.tensor_copy(kv_sb[:, m], kv_ps[:])
        k_sb = kv_sb[:, :, 0:INNER]
        v_sb = kv_sb[:, :, INNER:2 * INNER]

        # ---- dots: per-m fp16 mults, then add-tree over d ----
        prod = scratch.tile([T, M, INNER], CD, tag="prod")
        for m in range(M):
            nc.vector.tensor_mul(prod[:, m], q_cd[:], k_sb[:, m])
        pr = prod[:].rearrange("p m (h d) -> p (m h) d", d=DHEAD)
        widths = [32, 16, 8, 4, 2]
        cur = pr
        for li, w in enumerate(widths):
            nxt = scratch.tile([T, M * HEADS, w], CD, tag=f"dt{w}")
            eng = nc.gpsimd if li == 0 else nc.vector
            eng.tensor_add(nxt[:], cur[:, :, 0:w], cur[:, :, w:2 * w])
            cur = nxt
        dots = scratch.tile([T, M * HEADS], CD, tag="dots")
        nc.vector.tensor_add(dots[:].unsqueeze(2), cur[:, :, 0:1], cur[:, :, 1:2])

        # ---- softmax over m (no max subtraction; |dots*SCALE| is O(5)) ----
        e2d = scratch.tile([T, M * HEADS], CD, tag="e2d")
        nc.scalar.activation(e2d[:], dots[:], mybir.ActivationFunctionType.Exp,
                             scale=float(SCALE))
        s_sb = scratch.tile([T, HEADS], F32, tag="s")
        nc.vector.tensor_reduce(
            s_sb[:], e2d[:].rearrange("p (m h) -> p h m", h=HEADS),
            axis=mybir.AxisListType.X, op=mybir.AluOpType.add)
        rs = scratch.tile([T, HEADS], F32, tag="rs")
        nc.vector.reciprocal(rs[:], s_sb[:])
        # normalized weights e' = e / S, then replicate across d on gpsimd
        e_n = scratch.tile([T, M, HEADS], CD, tag="e_n")
        nc.vector.tensor_mul(
            e_n[:], e2d[:].rearrange("p (m h) -> p m h", h=HEADS),
            rs[:].unsqueeze(1).broadcast_to([T, M, HEADS]))
        e_rep = scratch.tile([T, M, HEADS, DHEAD], CD, tag="e_rep")
        nc.gpsimd.tensor_copy(e_rep[:],
                              e_n[:].unsqueeze(3).broadcast_to([T, M, HEADS, DHEAD]))

        # ---- weighted v: per-m fp16 mults, add-tree over m ----
        prod2 = scratch.tile([T, M, INNER], CD, tag="prod")
        e_rep_f = e_rep[:].rearrange("p m h d -> p m (h d)")
        for m in range(M):
            nc.vector.tensor_mul(prod2[:, m], v_sb[:, m], e_rep_f[:, m])
        mw = [8, 4, 2]
        curm = prod2[:]
        for li, w in enumerate(mw):
            nxt = scratch.tile([T, w, INNER], CD, tag=f"at{w}")
            eng = nc.gpsimd if li == 0 else nc.vector
            eng.tensor_add(nxt[:], curm[:, 0:w], curm[:, w:2 * w])
            curm = nxt
        av = scratch.tile([T, INNER], CD, tag="av")
        nc.vector.tensor_add(av[:].unsqueeze(1), curm[:, 0:1], curm[:, 1:2])

        # ---- out projection ----
        aoT_ps = q_psum.tile([128, 4, 128], CD, tag="q")
        for ci in range(4):
            nc.tensor.transpose(aoT_ps[:, ci], av[:, bass.ts(ci, 128)], ident_cd[:])
        aoT = work.tile([128, 4, 128], CD, tag="aoT")
        nc.any.tensor_copy(aoT[:], aoT_ps[:])

        o_ps = o_psum.tile([T, DIM], F32, tag="o")
        for ci in range(4):
            nc.tensor.matmul(o_ps[:], aoT[:, ci], wout_sb[:, ci],
                             start=(ci == 0), stop=False)
        nc.tensor.matmul(o_ps[:], ones_sb[:], bout_cd[:], start=False, stop=True)

        o_sb = stage.tile([T, DIM], F32, tag="o")
        nc.any.tensor_copy(o_sb[:], o_ps[:])
        nc.sync.dma_start(out_t[t], o_sb[:])


_NC_CACHE = {}


def get_nc():
    if "nc" not in _NC_CACHE:
        _NC_CACHE["nc"] = _build_nc()
    return _NC_CACHE["nc"]


def make_in_maps(x, y, W_q, W_kv, W_out, b_out):
    in_maps = []
    for i in range(NCORES):
        in_maps.append({
            "x": np.ascontiguousarray(x[i], dtype=np.float32),
            "y": np.ascontiguousarray(y[i], dtype=np.float32).reshape(N * M, DIM),
            "wq": np.ascontiguousarray(W_q, dtype=np.float32),
            "wkv": np.ascontiguousarray(W_kv, dtype=np.float32),
            "wout": np.ascontiguousarray(W_out, dtype=np.float32),
            "bout": np.ascontiguousarray(b_out, dtype=np.float32).reshape(1, DIM),
        })
    return in_maps


def kernel(x, y, W_q, W_kv, W_out, b_out):
    nc = get_nc()
    in_maps = make_in_maps(x, y, W_q, W_kv, W_out, b_out)
    res = run_bass_kernel_spmd(nc, in_maps, core_ids=list(range(NCORES)))
    return np.stack([res.results[i]["out"] for i in range(NCORES)]).astype(np.float32)


# revision 16
# speedup vs baseline: 2.5390x; 2.5390x over previous
"""Trainium2 Bass kernel for cross-attention (single query per position, m=16 context).

Reference computation (per batch b, position n):
  q = x @ W_q                      [n, 512] -> heads h=8, d=64
  k,v = y @ W_kv                   [n, m, 512] each
  dots[h,m] = (q_h . k_mh) / 8
  attn = softmax_m(dots)
  out = (sum_m attn * v) @ W_out + b_out

Sharding: data-parallel over batch (8 batches -> 8 NeuronCores), weights replicated.
"""

import numpy as np
from contextlib import ExitStack

import concourse.bass as bass
import concourse.bacc as bacc
import concourse.mybir as mybir
import concourse.tile as tile
from concourse.bass_utils import run_bass_kernel_spmd
from concourse.masks import make_identity

B, N, M, DIM = 8, 2048, 16, 256
HEADS, DHEAD, INNER = 8, 64, 512
SCALE = DHEAD**-0.5
NCORES = 8
T = 128          # positions per tile
NT = N // T      # 16 tiles per core

F32 = mybir.dt.float32
CD = mybir.dt.float16  # compute dtype: fp16 keeps 10 mantissa bits and 2x DVE modes


def _build_nc():
    nc = bacc.Bacc("TRN2", target_bir_lowering=False, debug=False, num_devices=NCORES)
    x = nc.dram_tensor("x", [N, DIM], F32, kind="ExternalInput").ap()
    y = nc.dram_tensor("y", [N * M, DIM], F32, kind="ExternalInput").ap()
    wq = nc.dram_tensor("wq", [DIM, INNER], F32, kind="ExternalInput").ap()
    wkv = nc.dram_tensor("wkv", [DIM, 2 * INNER], F32, kind="ExternalInput").ap()
    wout = nc.dram_tensor("wout", [INNER, DIM], F32, kind="ExternalInput").ap()
    bout = nc.dram_tensor("bout", [1, DIM], F32, kind="ExternalInput").ap()
    out = nc.dram_tensor("out", [N, DIM], F32, kind="ExternalOutput").ap()

    with tile.TileContext(nc) as tc:
        with ExitStack() as ctx:
            _body(ctx, tc, out, x, y, wq, wkv, wout, bout)
    nc.compile()
    return nc


def _body(ctx, tc, out, x, y, wq, wkv, wout, bout):
    nc = tc.nc
    consts = ctx.enter_context(tc.tile_pool(name="consts", bufs=1))
    stage = ctx.enter_context(tc.tile_pool(name="stage", bufs=2))
    ypool = ctx.enter_context(tc.tile_pool(name="ypool", bufs=2))
    ytp = ctx.enter_context(tc.tile_pool(name="ytp", bufs=1))
    work = ctx.enter_context(tc.tile_pool(name="work", bufs=2))
    scratch = ctx.enter_context(tc.tile_pool(name="scratch", bufs=1))
    tp_psum = ctx.enter_context(tc.tile_pool(name="tp_psum", bufs=2, space="PSUM"))
    kv_psum = ctx.enter_context(tc.tile_pool(name="kv_psum", bufs=2, space="PSUM"))
    q_psum = ctx.enter_context(tc.tile_pool(name="q_psum", bufs=1, space="PSUM"))
    o_psum = ctx.enter_context(tc.tile_pool(name="o_psum", bufs=1, space="PSUM"))

    ident = consts.tile([128, 128], F32, tag="ident")
    make_identity(nc, ident[:])
    ident_cd = consts.tile([128, 128], CD, tag="ident_cd")
    nc.any.tensor_copy(ident_cd[:], ident[:])

    # --- weights: [c, cols] with contraction chunked to 128 partitions ---
    def load_w(ap, n_chunks, cols, name):
        st = scratch.tile([128, n_chunks, cols], F32, tag="wstage")
        nc.sync.dma_start(st[:], ap.rearrange("(a p) i -> p a i", p=128))
        cd = consts.tile([128, n_chunks, cols], CD, tag=f"{name}_cd")
        nc.any.tensor_copy(cd[:], st[:])
        return cd

    wq_sb = load_w(wq, 2, INNER, "wq")

    # W_kv: keep k columns in (h,d) order; permute v columns to (d,h) order so the
    # attention-weight broadcast in the weighted-v multiply has stride-0 on a middle
    # dim (innermost stays step-1 -> DVE 2x packing).
    wkv_st = scratch.tile([128, 2, 2 * INNER], F32, tag="wstage")
    nc.sync.dma_start(wkv_st[:], wkv.rearrange("(a p) i -> p a i", p=128))
    wkv_sb = consts.tile([128, 2, 2 * INNER], CD, tag="wkv_cd")
    nc.any.tensor_copy(wkv_sb[:, :, 0:INNER], wkv_st[:, :, 0:INNER])
    nc.any.tensor_copy(
        wkv_sb[:, :, INNER:2 * INNER].rearrange("p a (d h) -> p a d h", h=HEADS),
        wkv_st[:, :, INNER:2 * INNER].rearrange("p a (h d) -> p a d h", d=DHEAD))

    # W_out rows permuted to the matching (d,h) order
    # Row (h*64+d) of W_out goes to permuted row (d*8+h): partition (d%16)*8+h,
    # chunk d//16. One strided DMA per head.
    wout_st = scratch.tile([128, 4, DIM], F32, tag="wstage")
    for h in range(HEADS):
        nc.sync.dma_start(
            wout_st[h:128:HEADS],
            wout[h * DHEAD:(h + 1) * DHEAD].rearrange("(a dd) f -> dd a f", a=4))
    wout_sb = consts.tile([128, 4, DIM], CD, tag="wout_cd")
    nc.any.tensor_copy(wout_sb[:], wout_st[:])

    # bias: added to the out-proj psum via ones[1,128].T @ bout[1,256]
    bout_f = consts.tile([1, DIM], F32, tag="bout_f")
    nc.sync.dma_start(bout_f[:], bout)
    ones_sb = consts.tile([1, 128], CD, tag="ones")
    nc.any.memset(ones_sb[:], 1.0)
    bout_cd = consts.tile([1, DIM], CD, tag="bout_cd")
    nc.any.tensor_copy(bout_cd[:], bout_f[:])

    x_t = x.rearrange("(t p) c -> t p c", p=T)
    y_t = y.rearrange("(t p m) c -> t p m c", p=T, m=M)
    out_t = out.rearrange("(t p) c -> t p c", p=T)

    for t in range(NT):
        # ---- x tile -> xT chunks [c, pos] ----
        x_sb = stage.tile([T, DIM], F32, tag="x")
        nc.sync.dma_start(x_sb[:], x_t[t])
        xT_ps = tp_psum.tile([128, 2, 128], F32, tag="tp")
        for ci in range(2):
            nc.tensor.transpose(xT_ps[:, ci], x_sb[:, bass.ts(ci, 128)], ident[:])
        xT = work.tile([128, 2, 128], CD, tag="xT")
        nc.any.tensor_copy(xT[:], xT_ps[:])

        # ---- q projection ----
        q_ps = q_psum.tile([T, INNER], F32, tag="q")
        for ci in range(2):
            nc.tensor.matmul(q_ps[:], xT[:, ci], wq_sb[:, ci],
                             start=(ci == 0), stop=(ci == 1))
        q_cd = work.tile([T, INNER], CD, tag="q_cd")
        nc.any.tensor_copy(q_cd[:], q_ps[:])

        # ---- y tile -> yT blocks, 4 transposes per psum tile, wide evac ----
        y_sb = ypool.tile([T, M, DIM], F32, tag="y")
        nc.sync.dma_start(y_sb[:], y_t[t])
        yT = ytp.tile([128, M, 2, 128], CD, tag="yT")
        y_flat = y_sb[:].rearrange("p m c -> p (m c)")
        for j in range(8):  # j covers (m, ci) pairs 4j..4j+3
            ps = tp_psum.tile([128, 4, 128], F32, tag="tp")
            for u in range(4):
                blk = 4 * j + u
                nc.tensor.transpose(ps[:, u], y_flat[:, bass.ts(blk, 128)], ident[:])
            nc.any.tensor_copy(
                yT[:].rearrange("p m c2 f -> p (m c2) f")[:, 4 * j:4 * j + 4], ps[:])

        # ---- kv projection per m into one [128,1024] psum; single evac ----
        kv_sb = work.tile([T, M, 2 * INNER], CD, tag="kv")
        for m in range(M):
            kv_ps = kv_psum.tile([T, 2 * INNER], F32, tag="kv")
            for ci in range(2):
                nc.tensor.matmul(kv_ps[:, 0:INNER], yT[:, m, ci],
                                 wkv_sb[:, ci, 0:INNER],
                                 start=(ci == 0), stop=(ci == 1))
            for ci in range(2):
                nc.tensor.matmul(kv_ps[:, INNER:2 * INNER], yT[:, m, ci],
                                 wkv_sb[:, ci, INNER:2 * INNER],
                                 start=(ci == 0), stop=(ci == 1))
            nc.any.tensor_copy(kv_sb[:, m], kv_ps[:])
        k_sb = kv_sb[:, :, 0:INNER]
        v_sb = kv_sb[:, :, INNER:2 * INNER]

        # ---- dots: one wide fp16 mult (q broadcast over m), then add-tree over d ----
        prod = scratch.tile([T, M, INNER], CD, tag="prod")
        nc.vector.tensor_mul(
            prod[:], k_sb,
            q_cd[:].unsqueeze(1).broadcast_to([T, M, INNER]))
        pr = prod[:].rearrange("p m (h d) -> p (m h) d", d=DHEAD)
        widths = [32, 16, 8, 4, 2]
        cur = pr
        for w in widths:
            nxt = scratch.tile([T, M * HEADS, w], CD, tag=f"dt{w}")
            nc.vector.tensor_add(nxt[:], cur[:, :, 0:w], cur[:, :, w:2 * w])
            cur = nxt
        dots = scratch.tile([T, M * HEADS], CD, tag="dots")
        nc.vector.tensor_add(dots[:].unsqueeze(2), cur[:, :, 0:1], cur[:, :, 1:2])

        # ---- softmax over m (no max subtraction; |dots*SCALE| is O(5)) ----
        e2d = scratch.tile([T, M * HEADS], CD, tag="e2d")
        nc.scalar.activation(e2d[:], dots[:], mybir.ActivationFunctionType.Exp,
                             scale=float(SCALE))
        s_sb = scratch.tile([T, HEADS], F32, tag="s")
        nc.vector.tensor_reduce(
            s_sb[:], e2d[:].rearrange("p (m h) -> p h m", h=HEADS),
            axis=mybir.AxisListType.X, op=mybir.AluOpType.add)
        rs = scratch.tile([T, HEADS], F32, tag="rs")
        nc.vector.reciprocal(rs[:], s_sb[:])
        # normalized weights e' = e / S
        e_n = scratch.tile([T, M, HEADS], CD, tag="e_n")
        nc.vector.tensor_mul(
            e_n[:], e2d[:].rearrange("p (m h) -> p m h", h=HEADS),
            rs[:].unsqueeze(1).broadcast_to([T, M, HEADS]))

        # ---- weighted v: one wide fp16 mult (v is (d,h)-ordered, so the weight
        # broadcast is stride-0 on the middle d dim), add-tree over m ----
        prod2 = scratch.tile([T, M, INNER], CD, tag="prod")
        nc.vector.tensor_mul(
            prod2[:].rearrange("p m (d h) -> p m d h", h=HEADS),
            v_sb.rearrange("p m (d h) -> p m d h", h=HEADS),
            e_n[:].unsqueeze(2).broadcast_to([T, M, DHEAD, HEADS]))
        mw = [8, 4, 2]
        curm = prod2[:]
        for w in mw:
            nxt = scratch.tile([T, w, INNER], CD, tag=f"at{w}")
            nc.vector.tensor_add(nxt[:], curm[:, 0:w], curm[:, w:2 * w])
            curm = nxt
        av = scratch.tile([T, INNER], CD, tag="av")
        nc.vector.tensor_add(av[:].unsqueeze(1), curm[:, 0:1], curm[:, 1:2])

        # ---- out projection ----
        aoT_ps = q_psum.tile([128, 4, 128], CD, tag="q")
        for ci in range(4):
            nc.tensor.transpose(aoT_ps[:, ci], av[:, bass.ts(ci, 128)], ident_cd[:])
        aoT = work.tile([128, 4, 128], CD, tag="aoT")
        nc.any.tensor_copy(aoT[:], aoT_ps[:])

        o_ps = o_psum.tile([T, DIM], F32, tag="o")
        for ci in range(4):
            nc.tensor.matmul(o_ps[:], aoT[:, ci], wout_sb[:, ci],
                             start=(ci == 0), stop=False)
        nc.tensor.matmul(o_ps[:], ones_sb[:], bout_cd[:], start=False, stop=True)

        o_sb = stage.tile([T, DIM], F32, tag="o")
        nc.any.tensor_copy(o_sb[:], o_ps[:])
        nc.sync.dma_start(out_t[t], o_sb[:])


_NC_CACHE = {}


def get_nc():
    if "nc" not in _NC_CACHE:
        _NC_CACHE["nc"] = _build_nc()
    return _NC_CACHE["nc"]


def make_in_maps(x, y, W_q, W_kv, W_out, b_out):
    in_maps = []
    for i in range(NCORES):
        in_maps.append({
            "x": np.ascontiguousarray(x[i], dtype=np.float32),
            "y": np.ascontiguousarray(y[i], dtype=np.float32).reshape(N * M, DIM),
            "wq": np.ascontiguousarray(W_q, dtype=np.float32),
            "wkv": np.ascontiguousarray(W_kv, dtype=np.float32),
            "wout": np.ascontiguousarray(W_out, dtype=np.float32),
            "bout": np.ascontiguousarray(b_out, dtype=np.float32).reshape(1, DIM),
        })
    return in_maps


def kernel(x, y, W_q, W_kv, W_out, b_out):
    nc = get_nc()
    in_maps = make_in_maps(x, y, W_q, W_kv, W_out, b_out)
    res = run_bass_kernel_spmd(nc, in_maps, core_ids=list(range(NCORES)))
    return np.stack([res.results[i]["out"] for i in range(NCORES)]).astype(np.float32)
